# revision 18
# baseline (speedup 1.0000x reference)
"""CoordinatesToSpikes on 8 TRN2 NeuronCores.

Reference semantics: times = T_EARLY + cv * (T_LATE - T_EARLY);
idx = round(times / DT); spikes = one-hot along a dense time axis of
length 1000 (each (b, c) pair scatters exactly one 1.0, so the scatter
is a pure one-hot materialization: out[b, t, c] = (idx[b, c] == t)).

Strategy (data-parallel over batch, 256 -> 8 x 32):
  - Host computes idx bit-exactly in fp32 (tiny: 64K elements) and a
    per-core diff tensor diff[p, f] = idx[p%32, f%256] - (p//32)*250
    - f//256 (1.25MB/core); partition p = tg*32 + b covers batch b,
    time-quarter tg. All values are exact small integers.
  - idx <= 800 always (times < 8e-4), so output rows 810..999 are
    guaranteed zeros: they are streamed from a zero tile (stride-0
    repeated source) starting right after the kernel preamble, with no
    data dependency -- filling the HBM window while diff loads.
  - Each of 25 chunks (10 time rows) is one DVE compare diff == 10*d
    producing the one-hot tile [128, 2560]; it is stored per
    time-quarter as [32, 2560] transfers (contiguous partition slices,
    10KB contiguous per partition). Chunks 6..24 skip tg=3 (rows
    810..999, already zero-filled). Stores rotate across three DGE
    queues (2 HWDGE rings + the GpSimd SWDGE ring).
  - Output is write-only, 32.8 MB per core => memory(store)-roofline;
    HBM stacks are shared pairwise (716 GB/s per 2 cores), so
    ~358 GB/s/core sustained and ~91.5us of unavoidable store time.
"""

import numpy as np
from contextlib import ExitStack

import concourse.bass as bass
import concourse.tile as tile
from concourse import bacc, mybir
from concourse.bass_utils import run_bass_kernel_spmd

F32 = mybir.dt.float32

B, C, SEQ = 256, 256, 1000
NCORES = 8
BSH = B // NCORES          # 32 batches per core
TG = 4                     # time quarters (partition = tg*32 + b)
TQ = SEQ // TG             # 250 time rows per quarter
TROWS = 10                 # time rows per chunk
ND = TQ // TROWS           # 25 chunks
FREE = TROWS * C           # 2560 free elements per tile (10KB)
ZROW = 810                 # rows >= ZROW are guaranteed zero (idx <= 800)
NZREP = (SEQ - ZROW) // TROWS  # 19 repeats of the 10-row zero tile

T_EARLY = np.float32(2e-06)
T_LATE_MINUS_EARLY = np.float32(0.0008 - 2e-06)
DT = np.float32(1e-06)

_compiled = None


def _build():
    nc = bacc.Bacc("TRN2", target_bir_lowering=False, debug=False,
                   num_devices=NCORES)
    diff_d = nc.dram_tensor("diff", [128, FREE], F32, kind="ExternalInput")
    out_d = nc.dram_tensor("out", [BSH, SEQ, C], F32, kind="ExternalOutput")

    def rows_ap(t0, nrows):
        # [32 batches (1MB stride), nrows*256 contiguous] view of
        # out[:, t0:t0+nrows, :]
        return out_d.ap()[:, t0:t0 + nrows, :].rearrange("b t c -> b (t c)")

    quart = FREE // 4
    with ExitStack() as ctx:
        tc = ctx.enter_context(tile.TileContext(nc))
        dpool = ctx.enter_context(tc.tile_pool(name="diff", bufs=1))
        zpool = ctx.enter_context(tc.tile_pool(name="zero", bufs=1))
        outp = ctx.enter_context(tc.tile_pool(name="outp", bufs=2))

        engines = [nc.sync, nc.scalar, nc.gpsimd]

        # diff load: four quarters, two per HWDGE ring.
        diff = dpool.tile([128, FREE], F32)
        for q in range(4):
            engines[q % 2].dma_start(
                diff[:, q * quart:(q + 1) * quart],
                diff_d.ap()[:, q * quart:(q + 1) * quart])

        # Zero-prefill rows 810..999: no data dependency, so these
        # stream on the HWDGE rings while diff loads and the first
        # group's compares run. Stride-0 source repeats the 10-row
        # zero tile.
        zero = zpool.tile([BSH, FREE], F32)
        nc.vector.memset(zero[:], 0.0)
        zsplit = [(ZROW, 9), (ZROW + 90, 10)]
        for i, (t0, reps) in enumerate(zsplit):
            engines[i].dma_start(
                out_d.ap()[:, t0:t0 + reps * TROWS, :].rearrange(
                    "b (r t) c -> b r (t c)", t=TROWS),
                zero[:].unsqueeze(1).broadcast_to((BSH, reps, FREE)))

        # Compare chunks in groups; each group's chunks land in one
        # wide buffer so per-tg stores are single 1.9-2.2MB transfers
        # (60-70KB contiguous per partition). Groups past chunk 5 skip
        # tg=3 (rows 810+ already zero-filled).
        groups = [(0, 6), (6, 6), (12, 6), (18, 7)]
        si = 0
        for g0, glen in groups:
            ob = outp.tile([128, glen * FREE], F32, tag="ob")
            for j in range(glen):
                nc.vector.tensor_scalar(
                    ob[:, j * FREE:(j + 1) * FREE], diff[:],
                    float(TROWS * (g0 + j)), None,
                    mybir.AluOpType.is_equal)
            ntg = TG if g0 == 0 else TG - 1
            for tg in range(ntg):
                engines[si % 3].dma_start(
                    rows_ap(tg * TQ + g0 * TROWS, glen * TROWS),
                    ob[tg * BSH:(tg + 1) * BSH, 0:glen * FREE])
                si += 1
    nc.compile()
    return nc


def _host_idx(coordinate_values: np.ndarray) -> np.ndarray:
    """Bit-exact fp32 mirror of the reference index computation."""
    cv = np.ascontiguousarray(coordinate_values, dtype=np.float32)
    times = T_EARLY + cv * T_LATE_MINUS_EARLY
    return np.rint(times / DT).astype(np.float32)


def _in_maps(coordinate_values: np.ndarray) -> list[dict]:
    idxf = _host_idx(coordinate_values)                      # (256, 256)
    p = np.arange(128)
    base = ((p // BSH) * TQ)[:, None] + np.repeat(
        np.arange(TROWS), C)[None, :]                        # (128, 2560)
    maps = []
    for m in range(NCORES):
        shard = idxf[m * BSH:(m + 1) * BSH]                  # (32, 256)
        tiled = np.tile(shard[p % BSH], (1, TROWS))          # (128, 2560)
        maps.append({"diff": (tiled - base).astype(np.float32)})
    return maps


def kernel(coordinate_values: np.ndarray) -> np.ndarray:
    global _compiled
    if _compiled is None:
        _compiled = _build()
    res = run_bass_kernel_spmd(
        _compiled, _in_maps(coordinate_values),
        core_ids=list(range(NCORES)))
    return np.concatenate([r["out"] for r in res.results], axis=0)


# revision 20
# speedup vs baseline: 1.7236x; 1.7236x over previous
"""CoordinatesToSpikes on 8 TRN2 NeuronCores.

Reference semantics: times = T_EARLY + cv * (T_LATE - T_EARLY);
idx = round(times / DT); spikes = one-hot along a dense time axis of
length 1000 (each (b, c) pair scatters exactly one 1.0, so the scatter
is a pure one-hot materialization: out[b, t, c] = (idx[b, c] == t)).

The module constants bound the spike support: times/DT <= 800.003 for
any cv in [0, 1], so idx is always in [2, 800] and rows 801..999 are
structurally zero for every possible input. The device therefore
materializes only the active band rows 0..839 (840 = 4*210 keeps the
uniform-partition-stride store shape); the host pads rows 840..999
with zeros during the required gather/unshard step.

Strategy (data-parallel over batch, 256 -> 8 x 32):
  - Host computes idx bit-exactly in fp32 (tiny: 64K elements) and a
    per-core diff tensor diff[p, f] = idx[p//4, f%256] - (p%4)*210
    - f//256 (1.25MB/core). All values are exact small integers.
  - On device, SBUF partition p covers batch b = p//4, time-quarter
    tg = p%4 (210 rows each) of the active band, so every partition's
    slice of the output is one contiguous 210KB DRAM range -> 10KB DMA
    descriptors across all 128 partitions. (1KB descriptors cap a
    single HWDGE ring at ~115 GB/s; 32-partition store shapes collapse
    ring throughput; [128 x 10KB] runs at the full SDMA rate.)
  - Each of 21 chunks (10 time rows) is one DVE compare diff == 10*d
    producing the one-hot tile [128, 2560], DMA-stored as a 1.25MB
    transfer, rotating across three DGE queues (2 HWDGE rings + the
    GpSimd SWDGE ring). The diff load is split into four quarters on
    the two HWDGE rings and chunk 0 is computed/stored as four column
    pieces so the store stream starts as early as possible.
  - Output band is write-only, 27.5 MB per core => memory roofline;
    HBM stacks are shared pairwise (716 GB/s per 2 cores), so
    ~358 GB/s/core sustained: ~77us of unavoidable store time.
"""

import numpy as np
from contextlib import ExitStack

import concourse.bass as bass
import concourse.tile as tile
from concourse import bacc, mybir
from concourse.bass_utils import run_bass_kernel_spmd

F32 = mybir.dt.float32

B, C, SEQ = 256, 256, 1000
NCORES = 8
BSH = B // NCORES          # 32 batches per core
TACT = 840                 # active band: idx <= 800 < 840, 840 = 4*210
TG = 4                     # time quarters per batch (partition = b*4+tg)
TQ = TACT // TG            # 210 active rows per quarter
TROWS = 10                 # time rows per chunk
ND = TQ // TROWS           # 21 chunks
FREE = TROWS * C           # 2560 free elements per tile (10KB)

T_EARLY = np.float32(2e-06)
T_LATE_MINUS_EARLY = np.float32(0.0008 - 2e-06)
DT = np.float32(1e-06)

_compiled = None


def _build():
    nc = bacc.Bacc("TRN2", target_bir_lowering=False, debug=False,
                   num_devices=NCORES)
    diff_d = nc.dram_tensor("diff", [128, FREE], F32, kind="ExternalInput")
    out_d = nc.dram_tensor("out", [BSH, TACT, C], F32, kind="ExternalOutput")
    # [128 partitions (b,tg) @ 210KB stride, 21 chunks, 2560 contiguous]
    out_v = out_d.ap().rearrange(
        "b (tg d t) c -> (b tg) d (t c)", tg=TG, d=ND, t=TROWS)

    quart = FREE // 4
    with ExitStack() as ctx:
        tc = ctx.enter_context(tile.TileContext(nc))
        dpool = ctx.enter_context(tc.tile_pool(name="diff", bufs=1))
        outp = ctx.enter_context(tc.tile_pool(name="outp", bufs=10))

        # Load diff in four quarters, two per HWDGE ring (the gpsimd
        # SWDGE ring has ~1us extra first-byte latency — stores only),
        # so the first chunk-0 piece can start as early as possible.
        engines = [nc.sync, nc.scalar, nc.gpsimd]
        diff = dpool.tile([128, FREE], F32)
        for q in range(4):
            engines[q % 2].dma_start(
                diff[:, q * quart:(q + 1) * quart],
                diff_d.ap()[:, q * quart:(q + 1) * quart])

        # Chunk 0 is computed/stored as four column pieces, each gated
        # only on its own quarter of the load (column slices of the
        # chunk stay contiguous per partition in DRAM); remaining chunks
        # go full-width. Stores rotate across the three DGE queues.
        for q in range(4):
            oq = outp.tile([128, quart], F32, tag="piece")
            nc.vector.tensor_scalar(
                oq[:], diff[:, q * quart:(q + 1) * quart], 0.0, None,
                mybir.AluOpType.is_equal)
            engines[q % 3].dma_start(
                out_v[:, 0, q * quart:(q + 1) * quart], oq[:])

        for d in range(1, ND):
            ot = outp.tile([128, FREE], F32)
            nc.vector.tensor_scalar(
                ot[:], diff[:], float(TROWS * d), None,
                mybir.AluOpType.is_equal)
            engines[d % 3].dma_start(out_v[:, d, :], ot[:])
    nc.compile()
    return nc


def _host_idx(coordinate_values: np.ndarray) -> np.ndarray:
    """Bit-exact fp32 mirror of the reference index computation."""
    cv = np.ascontiguousarray(coordinate_values, dtype=np.float32)
    times = T_EARLY + cv * T_LATE_MINUS_EARLY
    return np.rint(times / DT).astype(np.float32)


def _in_maps(coordinate_values: np.ndarray) -> list[dict]:
    idxf = _host_idx(coordinate_values)                      # (256, 256)
    p = np.arange(128)
    base = ((p % TG) * TQ)[:, None] + np.repeat(
        np.arange(TROWS), C)[None, :]                        # (128, 2560)
    maps = []
    for m in range(NCORES):
        shard = idxf[m * BSH:(m + 1) * BSH]                  # (32, 256)
        tiled = np.tile(shard[p // TG], (1, TROWS))          # (128, 2560)
        maps.append({"diff": (tiled - base).astype(np.float32)})
    return maps


def kernel(coordinate_values: np.ndarray) -> np.ndarray:
    global _compiled
    if _compiled is None:
        _compiled = _build()
    res = run_bass_kernel_spmd(
        _compiled, _in_maps(coordinate_values),
        core_ids=list(range(NCORES)))
    # Gather/unshard: concatenate batch shards and pad the structurally
    # zero rows 840..999 (idx <= 800 for any input by module constants).
    full = np.zeros((B, SEQ, C), dtype=np.float32)
    for m in range(NCORES):
        full[m * BSH:(m + 1) * BSH, 0:TACT, :] = res.results[m]["out"]
    return full


# revision 21
# speedup vs baseline: 1.8140x; 1.0524x over previous
"""CoordinatesToSpikes on 8 TRN2 NeuronCores.

Reference semantics: times = T_EARLY + cv * (T_LATE - T_EARLY);
idx = round(times / DT); spikes = one-hot along a dense time axis of
length 1000 (each (b, c) pair scatters exactly one 1.0, so the scatter
is a pure one-hot materialization: out[b, t, c] = (idx[b, c] == t)).

The module constants bound the spike support: times/DT <= 800.003 for
any cv in [0, 1], so idx is always in [2, 800] and rows 801..999 are
structurally zero for every possible input. The device therefore
materializes only the active band rows 0..839 (840 = 4*210 keeps the
uniform-partition-stride store shape); the host pads rows 840..999
with zeros during the required gather/unshard step.

Strategy (data-parallel over batch, 256 -> 8 x 32):
  - Host computes idx bit-exactly in fp32 (tiny: 64K elements) and a
    per-core diff tensor diff[p, f] = idx[p//4, f%256] - (p%4)*210
    - f//256 (1.25MB/core). All values are exact small integers.
  - On device, SBUF partition p covers batch b = p//4, time-quarter
    tg = p%4 (210 rows each) of the active band, so every partition's
    slice of the output is one contiguous 210KB DRAM range -> 10KB DMA
    descriptors across all 128 partitions. (1KB descriptors cap a
    single HWDGE ring at ~115 GB/s; 32-partition store shapes collapse
    ring throughput; [128 x 10KB] runs at the full SDMA rate.)
  - Each of 21 chunks (10 time rows) is one DVE compare diff == 10*d
    producing the one-hot tile [128, 2560], DMA-stored as a 1.25MB
    transfer, rotating across three DGE queues (2 HWDGE rings + the
    GpSimd SWDGE ring). The diff load is split into four quarters on
    the two HWDGE rings and chunk 0 is computed/stored as four column
    pieces so the store stream starts as early as possible.
  - Output band is write-only, 27.5 MB per core => memory roofline;
    HBM stacks are shared pairwise (716 GB/s per 2 cores), so
    ~358 GB/s/core sustained: ~77us of unavoidable store time.
"""

import numpy as np
from contextlib import ExitStack

import concourse.bass as bass
import concourse.tile as tile
from concourse import bacc, mybir
from concourse.bass_utils import run_bass_kernel_spmd

F32 = mybir.dt.float32

B, C, SEQ = 256, 256, 1000
NCORES = 8
BSH = B // NCORES          # 32 batches per core
TACT = 820                 # active band: idx <= 800 < 820, 820 = 4*205
TG = 4                     # time quarters per batch (partition = b*4+tg)
TQ = TACT // TG            # 205 active rows per quarter
TROWS = 5                  # time rows per chunk
ND = TQ // TROWS           # 41 chunks
FREE = TROWS * C           # 2560 free elements per tile (10KB)

T_EARLY = np.float32(2e-06)
T_LATE_MINUS_EARLY = np.float32(0.0008 - 2e-06)
DT = np.float32(1e-06)

_compiled = None


def _build():
    nc = bacc.Bacc("TRN2", target_bir_lowering=False, debug=False,
                   num_devices=NCORES)
    diff_d = nc.dram_tensor("diff", [128, FREE], F32, kind="ExternalInput")
    out_d = nc.dram_tensor("out", [BSH, TACT, C], F32, kind="ExternalOutput")
    # [128 partitions (b,tg) @ 210KB stride, 21 chunks, 2560 contiguous]
    out_v = out_d.ap().rearrange(
        "b (tg d t) c -> (b tg) d (t c)", tg=TG, d=ND, t=TROWS)

    quart = FREE // 4
    with ExitStack() as ctx:
        tc = ctx.enter_context(tile.TileContext(nc))
        dpool = ctx.enter_context(tc.tile_pool(name="diff", bufs=1))
        outp = ctx.enter_context(tc.tile_pool(name="outp", bufs=10))

        # Load diff in four quarters, two per HWDGE ring (the gpsimd
        # SWDGE ring has ~1us extra first-byte latency — stores only),
        # so the first chunk-0 piece can start as early as possible.
        engines = [nc.sync, nc.scalar, nc.gpsimd]
        diff = dpool.tile([128, FREE], F32)
        for q in range(4):
            engines[q % 2].dma_start(
                diff[:, q * quart:(q + 1) * quart],
                diff_d.ap()[:, q * quart:(q + 1) * quart])

        # Chunk 0 is computed/stored as four column pieces, each gated
        # only on its own quarter of the load (column slices of the
        # chunk stay contiguous per partition in DRAM); remaining chunks
        # go full-width. Stores rotate across the three DGE queues.
        for q in range(4):
            oq = outp.tile([128, quart], F32, tag="piece")
            nc.vector.tensor_scalar(
                oq[:], diff[:, q * quart:(q + 1) * quart], 0.0, None,
                mybir.AluOpType.is_equal)
            engines[q % 3].dma_start(
                out_v[:, 0, q * quart:(q + 1) * quart], oq[:])

        for d in range(1, ND):
            ot = outp.tile([128, FREE], F32)
            nc.vector.tensor_scalar(
                ot[:], diff[:], float(TROWS * d), None,
                mybir.AluOpType.is_equal)
            engines[d % 3].dma_start(out_v[:, d, :], ot[:])
    nc.compile()
    return nc


def _host_idx(coordinate_values: np.ndarray) -> np.ndarray:
    """Bit-exact fp32 mirror of the reference index computation."""
    cv = np.ascontiguousarray(coordinate_values, dtype=np.float32)
    times = T_EARLY + cv * T_LATE_MINUS_EARLY
    return np.rint(times / DT).astype(np.float32)


def _in_maps(coordinate_values: np.ndarray) -> list[dict]:
    idxf = _host_idx(coordinate_values)                      # (256, 256)
    p = np.arange(128)
    base = ((p % TG) * TQ)[:, None] + np.repeat(
        np.arange(TROWS), C)[None, :]                        # (128, 2560)
    maps = []
    for m in range(NCORES):
        shard = idxf[m * BSH:(m + 1) * BSH]                  # (32, 256)
        tiled = np.tile(shard[p // TG], (1, TROWS))          # (128, 2560)
        maps.append({"diff": (tiled - base).astype(np.float32)})
    return maps


def kernel(coordinate_values: np.ndarray) -> np.ndarray:
    global _compiled
    if _compiled is None:
        _compiled = _build()
    res = run_bass_kernel_spmd(
        _compiled, _in_maps(coordinate_values),
        core_ids=list(range(NCORES)))
    # Gather/unshard: concatenate batch shards and pad the structurally
    # zero rows 840..999 (idx <= 800 for any input by module constants).
    full = np.zeros((B, SEQ, C), dtype=np.float32)
    for m in range(NCORES):
        full[m * BSH:(m + 1) * BSH, 0:TACT, :] = res.results[m]["out"]
    return full
